# revision 9
# baseline (speedup 1.0000x reference)
"""Trainium2 Bass kernel for per-sample covariance pooling + fc + L2 norm.

Reference computation (per sample b of B=32):
    xc  = x[b] - mean(x[b], axis=0)            # x[b]: [N=20000, D=64]
    cov = xc.T @ xc / (N-1)                    # [64, 64]
    out = normalize(cov.flatten() @ W + b)     # [256]

Kernel formulation (scale/norm invariant):
    G = x.T @ x on device (fp8 x);  R = s s^T/(N(N-1)) + I on HOST (exact)
    feat = 32*(G/(N-1) - R)                    # = 32*(cov - I)
    out = normalize(feat @ (8 W) + 256*(b + rowsum_diag(W)))   # identical

The identity shift keeps cov's ~1.0 diagonal (which would amplify fp8's
2^-4 mantissa into a 3-4% output error) inside the full-precision bias
row; the residual feat/W8 coupling is ~3e-3. All x / W / feat traffic
is fp8 e4m3 (TRN flavor). The host computes the tiny per-sample mean
outer product exactly, so the device G pass is a pure chunked
self-matmul: chunks are 64 cols (no ones column) and pair into 128-col
blocks -- either one FWL matmul per pair (G_MODE=fwl: cross blocks
discarded, diagonal blocks summed by DVE) or one fp8 DoubleRow matmul
per pair (G_MODE=dr: 2 rhs cols/cycle). Sharding: data-parallel over
batch, 4 samples/core.
"""

import os
import sys

import numpy as np
import ml_dtypes

for _p in ("/opt/trn_rl_repo",):
    if _p not in sys.path:
        sys.path.append(_p)

# Problem shapes (hardcoded per contract).
B, N, D, OUT = 32, 20000, 64, 256
NCORES = 8
BPC = B // NCORES            # samples per core
P = 128                      # SBUF partitions / matmul contraction tile
NCH = 158                    # 128-row chunks after padding (even)
NPR = NCH // 2               # 79 chunk pairs
NPAD = NCH * P               # 20224 rows after zero padding
KC = (D * D) // P            # 32 fc contraction chunks
SC_FEAT = 32.0               # feat scale (cancelled by the L2 norm)
# x DMA schedule per sample: (pair offset, pairs per DMA); 20 pairs = 320 KB
X_TILES = [(0, 20), (20, 20), (40, 20), (60, 19)]
W_SLICES = 8                 # 256 KB fp16 W slices on ring 1
FILL_PER_TILE = 1            # HAM-warming dummy matmuls per x tile
G_MODE = os.environ.get("G_MODE", "dr")  # "dr" (DoubleRow) | "fwl"

_CACHE = {}


def _split_drain_and_barrier(self, tick_clock, wait_clock):
    """Replacement for TileContext._drain_and_barrier emitting one drain per
    sem wait: this walrus vintage rejects >1 sync-wait per instruction."""
    import bass_rust
    import concourse.mybir as mybir

    drain_bi = self.nc.sync.drain()
    inst = drain_bi.ins
    wait_clock.add_sem_waits(
        drain_bi.ins, bass_rust.ScopedClock({None: tick_clock.global_clock})
    )
    waits = list(inst.sync_info.on_wait) if inst.sync_info else []
    if len(waits) > 1:
        # one pure sem-wait NoOp per extra wait (cheaper than extra drains)
        inst.sync_info = mybir.SyncInfo(on_wait=waits[:1], on_update=[])
        for w in waits[1:]:
            nop = mybir.InstNoOp(
                name=f"tailwait-{w.ant_name}",
                engine=mybir.EngineType.SP,
                sync_info=mybir.SyncInfo(on_wait=[w], on_update=[]),
                bass_nofuse=True,
            )
            self.nc.sync.add_instruction(nop)

    self.nc.all_engine_barrier()
    assert self.sems is not None
    popped = self.nc._tile_sem_poison_stack.pop()
    assert popped is self._sem_poison
    self.nc.clear_and_free_semaphores(list(self.sems.allocated().values()))
    self.nc.all_engine_barrier()


def _build_nc():
    import types

    import concourse.bass as bass
    import concourse.mybir as mybir
    from concourse.tile import TileContext

    dt = mybir.dt
    AF = mybir.ActivationFunctionType
    DR = mybir.MatmulPerfMode.DoubleRow
    nc = bass.Bass()

    xin = nc.dram_tensor("xin", [BPC, NCH * D * P], dt.float8e4, kind="ExternalInput")
    win = nc.dram_tensor("win", [P, KC * OUT], dt.float16, kind="ExternalInput")
    # cols 0:OUT: 256*(b + diag-rowsum of W); cols OUT:OUT+BPC: ones (same
    # row -- matmul operands must start at partition 0/32/64)
    bin_ = nc.dram_tensor("bin", [1, OUT + BPC], dt.float16, kind="ExternalInput")
    # rim[d, bb, e] = 32*(s_bb[d] s_bb[e]/(N(N-1)) + I[d,e]) (host, exact)
    rim = nc.dram_tensor("rim", [D, BPC * D], dt.float32, kind="ExternalInput")
    yout = nc.dram_tensor("yout", [BPC, OUT], dt.float32, kind="ExternalOutput")

    # The walrus vintage here supports only ONE sync-wait on data
    # instructions (DMA pseudo ops, TensorCopy, ...). The whole kernel is
    # structured so every emitted instruction needs at most one wait:
    #  - x tiles get one pool slot per DMA (no slot reuse -> 0 waits)
    #  - per-sample psum G tiles are not reused (gpsum bufs=BPC)
    #  - all cross-engine joins are relayed so same-engine waits merge
    #  - PE "observes" the W/bias DMA lanes early via dummy matmuls and
    #    the bias matmul; DVE observes the rim DMA, so the per-sample
    #    feat writes and the fc matmuls only carry one fresh wait each.
    tc = TileContext(nc)
    tc._drain_and_barrier = types.MethodType(_split_drain_and_barrier, tc)
    with tc:
        with (
            tc.tile_pool(name="const", bufs=1) as cpool,
            tc.tile_pool(name="xp", bufs=len(X_TILES) * BPC) as xpool,
            tc.tile_pool(name="small", bufs=2) as spool,
            tc.tile_pool(name="featp", bufs=1) as fpool,
            tc.tile_pool(name="gpsum", bufs=BPC, space="PSUM") as gpool,
            tc.tile_pool(name="opsum", bufs=1, space="PSUM") as opool,
        ):
            # Small constants ride the gpsimd SWDGE ring so the two HWDGE
            # rings (SP + ACT) stay clear for the x/W stream.
            w_sb = cpool.tile([P, KC * OUT], dt.float16)
            bias_sb = cpool.tile([1, OUT + BPC], dt.float16)
            rim_sb = cpool.tile([D, BPC, D], dt.float32)
            nc.gpsimd.dma_start(out=bias_sb[:], in_=bin_[:])
            nc.gpsimd.dma_start(
                out=rim_sb[:], in_=rim[:].rearrange("p (b f) -> p b f", b=BPC)
            )

            ring = [nc.sync, nc.scalar]
            rr = [0]

            def ring_dma(out, in_, force=None):
                r = force if force is not None else rr[0] % 2
                if force is None:
                    rr[0] += 1
                ring[r].dma_start(out=out, in_=in_)

            WSL = KC * OUT // W_SLICES
            wq = list(range(W_SLICES))  # pending W slice ids

            def issue_w_slices(k):
                for _ in range(k):
                    if wq:
                        c = wq.pop(0)
                        ring_dma(
                            w_sb[:, c * WSL : (c + 1) * WSL],
                            win[:, c * WSL : (c + 1) * WSL],
                            force=1,
                        )

            # DVE observes the rim DMA lane once, so each sample's feat
            # writes only need their PE wait.
            obs = spool.tile([1, 1], dt.float32, tag="obs")
            nc.vector.tensor_copy(obs[:], rim_sb[0:1, 0:1, 0])

            # feat_sb[p, c, bb] = flattened 32*(C - I) for sample bb in fc
            # chunk layout: element k = c*128 + p of C.flatten(). Using C's
            # symmetry, k = d*64+e maps to (p = (d%2)*64 + e, c = d//2): no
            # transpose needed. fp16: the cov residual here is ~10x larger
            # than the CLT estimate, so fp8 feat/W couple a 2e-2 error into
            # the output -- measured; only x itself tolerates fp8.
            feat_sb = fpool.tile([P, KC, BPC], dt.float16)

            po = opool.tile([BPC, OUT], dt.float32)
            pdum = opool.tile([1, 512], dt.float32, tag="pdum")

            # Pre-warm the PE clock gate (HAM) with dummy matmuls on a memset
            # tile while the first x tile is still in flight: the gate needs
            # ~3.4 us of sustained activity to lift the cold throttle.
            dumsrc = cpool.tile([P, 512], dt.bfloat16)
            nc.vector.memset(dumsrc[:], 0.5)
            for _ in range(8):
                nc.tensor.matmul(
                    pdum[:], lhsT=dumsrc[:, 0:1], rhs=dumsrc[:, 0:512],
                    start=True, stop=True,
                )

            def do_sample(bb):
                if G_MODE == "dr":
                    pg = gpool.tile([D, D], dt.float32, tag="pg")
                else:
                    pg = gpool.tile([P, P], dt.float32, tag="pg")
                for ti, (p0, npr) in enumerate(X_TILES):
                    xt = xpool.tile([P, npr * P], dt.float8e4, tag="xt")
                    # sample 0 serial on ring 0: halving bandwidth across two
                    # rings would delay the first tile (and PE start)
                    ring_dma(
                        xt[:],
                        xin[bb, p0 * P * P : (p0 + npr) * P * P].rearrange(
                            "(p f) -> p f", p=P
                        ),
                        force=0 if bb == 0 else None,
                    )
                    if bb == 1:
                        issue_w_slices(1)
                    for j in range(npr):
                        blk = xt[:, j * P : (j + 1) * P]
                        if G_MODE == "dr":
                            ch = blk.rearrange("p (two f) -> p two f", two=2)
                            nc.tensor.matmul(
                                pg[:], lhsT=ch, rhs=ch,
                                start=(p0 + j == 0), stop=(p0 + j == NPR - 1),
                                perf_mode=DR,
                            )
                        else:
                            nc.tensor.matmul(
                                pg[:], lhsT=blk, rhs=blk,
                                start=(p0 + j == 0), stop=(p0 + j == NPR - 1),
                            )
                    # HAM-warming filler: keeps the PE activity monitor from
                    # dropping the clock during DMA slack. No new deps.
                    for _ in range(FILL_PER_TILE):
                        nc.tensor.matmul(
                            pdum[:], lhsT=xt[:, 0:1], rhs=xt[:, 0:512],
                            start=True, stop=True,
                        )
                # feat = 32*(G/(N-1) - R) with R host-computed; strided even/
                # odd column writes build the fc chunk layout directly.
                if G_MODE == "dr":
                    gs = pg[:]
                else:
                    gs = spool.tile([D, D], dt.float32, tag="gsum")
                    nc.vector.tensor_add(gs[:], pg[0:D, 0:D], pg[D:P, D:P])
                    gs = gs[:]
                ge = gs.rearrange("p (c two) -> p c two", two=2)
                re = rim_sb[:, bb, :].rearrange("p (c two) -> p c two", two=2)
                nc.vector.scalar_tensor_tensor(
                    feat_sb[0:D, :, bb], ge[:, :, 0], SC_FEAT / (N - 1.0),
                    re[:, :, 0], op0=mybir.AluOpType.mult,
                    op1=mybir.AluOpType.subtract,
                )
                nc.vector.scalar_tensor_tensor(
                    feat_sb[D:P, :, bb], ge[:, :, 1], SC_FEAT / (N - 1.0),
                    re[:, :, 1], op0=mybir.AluOpType.mult,
                    op1=mybir.AluOpType.subtract,
                )
                # keep the PE array warm across the sample-boundary stall
                for _ in range(0 if bb == 0 else 2):
                    nc.tensor.matmul(
                        pdum[:], lhsT=xt[:, 0:1], rhs=xt[:, 0:512],
                        start=True, stop=True,
                    )

            do_sample(0)
            do_sample(1)
            do_sample(2)
            issue_w_slices(W_SLICES)  # any stragglers
            # PE observes every W slice's DMA lane (all slices complete
            # during sample 2's stream; no PE stall here) so the fc matmuls
            # later need no DMA waits of their own.
            for c in range(W_SLICES):
                nc.tensor.matmul(
                    pdum[0:1, 0:1], lhsT=w_sb[0:1, c * WSL : c * WSL + 1],
                    rhs=w_sb[0:1, c * WSL : c * WSL + 1],
                    start=True, stop=True,
                )
            # Open the fc accumulation with the bias row: po = 1 * bias'.
            nc.tensor.matmul(
                po[:], lhsT=bias_sb[0:1, OUT : OUT + BPC], rhs=bias_sb[0:1, 0:OUT],
                start=True, stop=False,
            )
            do_sample(3)

            # fc: out[bb, o] = bias'[o] + sum_k feat[k, bb] * W[k, o]
            for c in range(KC):
                nc.tensor.matmul(
                    po[:],
                    lhsT=feat_sb[:, c, :],
                    rhs=w_sb[:, c * OUT : (c + 1) * OUT],
                    start=False,
                    stop=(c == KC - 1),
                )

            # L2 normalize rows: out = po / ||po||. ACT fuses square+rowsum
            # in one op; the tiny sqrt stays on ACT (no extra engine hop).
            sq = spool.tile([BPC, OUT], dt.float32, tag="sq")
            ss = spool.tile([BPC, 1], dt.float32, tag="ss")
            nc.scalar.activation(sq[:], po[:], AF.Square, accum_out=ss[:])
            nrm = spool.tile([BPC, 1], dt.float32, tag="nrm")
            nc.scalar.activation(nrm[:], ss[:], AF.Sqrt)
            inv = spool.tile([BPC, 1], dt.float32, tag="inv")
            nc.vector.reciprocal(inv[:], nrm[:])
            out_sb = spool.tile([BPC, OUT], dt.float32, tag="osb")
            nc.vector.tensor_scalar_mul(out_sb[:], po[:], inv[:])
            nc.gpsimd.dma_start(out=yout[:], in_=out_sb[:])

    return nc


def _get_nc():
    if "nc" not in _CACHE:
        _CACHE["nc"] = _build_nc()
    return _CACHE["nc"]


def _pack_inputs(x, W, b):
    x = np.asarray(x, dtype=np.float32)
    W = np.asarray(W, dtype=np.float32)
    b = np.asarray(b, dtype=np.float32)
    f8 = ml_dtypes.float8_e4m3

    aug = np.zeros((B, NPAD, D), dtype=f8)
    aug[:, :N, :] = x.astype(f8)
    # row n = chunk i*128 + partition p -> [B, p, i, D], then regroup into
    # DMA tiles so each dma_start reads one fully contiguous DRAM extent:
    # [B][tile][p][npr*128]
    augT = aug.reshape(B, NCH, P, D).transpose(0, 2, 1, 3)  # [B,P,NCH,D]
    parts = []
    for (p0, npr) in X_TILES:
        blk = augT[:, :, 2 * p0 : 2 * (p0 + npr), :].reshape(B, P, npr * P)
        parts.append(blk.reshape(B, P * npr * P))
    xcat = np.ascontiguousarray(np.concatenate(parts, axis=1))

    wp = np.ascontiguousarray(
        W.reshape(KC, P, OUT).transpose(1, 0, 2)
    ).reshape(P, KC * OUT).astype(np.float16)
    bias_new = SC_FEAT * (b + W[(D + 1) * np.arange(D)].sum(axis=0))
    bp = np.concatenate([bias_new, np.ones(BPC, np.float32)]).astype(
        np.float16
    ).reshape(1, OUT + BPC)

    # Host-exact mean correction + identity shift, in feat units:
    # rim[b] = 32*(s s^T/(N(N-1)) + I), laid out [D, BPC, D] per core.
    s = x.sum(axis=1, dtype=np.float64)  # [B, D]
    rims = SC_FEAT * (
        np.einsum("bd,be->bde", s, s) / (N * (N - 1.0))
        + np.eye(D, dtype=np.float64)[None]
    ).astype(np.float32)  # [B, D, D]

    return [
        {
            "xin": np.ascontiguousarray(xcat[c * BPC : (c + 1) * BPC]),
            "win": wp,
            "bin": bp,
            "rim": np.ascontiguousarray(
                rims[c * BPC : (c + 1) * BPC].transpose(1, 0, 2)
            ).reshape(D, BPC * D),
        }
        for c in range(NCORES)
    ]


def run(x, W, b, trace=False):
    from concourse.bass_utils import run_bass_kernel_spmd

    nc = _get_nc()
    in_maps = _pack_inputs(x, W, b)
    res = run_bass_kernel_spmd(nc, in_maps, list(range(NCORES)), trace=trace)
    out = np.concatenate(
        [res.results[c]["yout"] for c in range(NCORES)], axis=0
    ).astype(np.float32)
    return out, res


def kernel(x, W, b):
    out, _ = run(x, W, b, trace=False)
    return out
